# revision 51
# baseline (speedup 1.0000x reference)
"""AttentionPooling (segment softmax-pool) Trainium2 kernel, 8 NeuronCores.

Sharding: each core owns 32 consecutive segments (contiguous node range since
`batch` is sorted); the 32 segments split into G=4 groups of spg=8 segments
whose node ranges are padded to a fixed gmax so all cores/groups run one
static graph.  All segment reductions are core-local; no collectives.

Math: the reference computes att = exp(s - mx)/(seg(exp(s - mx)) + 1e-8)
with mx = seg(s) (a scatter-add "max" stabilizer).  We use the exact
factored form
    out[seg] = (sum_n e^{s_n} x_n) / (sum_n e^{s_n} + 1e-8 * e^{mx_seg})
so pass-2 runs on unnormalized weights w = e^s and the 1/denominator is a
per-segment scale folded into the final [spg, 512] psum->sbuf copy.  No
per-node max gather, no per-node normalize.

Per-core, per-group pipeline:
  pass 1 (feature-major, fp8 e4m3 DoubleRow / fp32 accum):
          h = tanh(W1^T @ x^T) ; scores via W2 "column-variant" matmuls that
          accumulate tile f's score row into PSUM ROW f of a per-group score
          bank (weights pre-scaled x64 into fp8-normal range, undone later)
  middle: score bank -> SBUF -> PE-transposed ([FA,128] blocks -> [128,FA])
          into node-major [128, 4, F] (the x_fm node permutation makes chunk
          c = j*F + f land exactly there, j = tile-column block); w = exp(s),
          one-hot partial reduce (DVE) + PE column-sum (ones as the stationary
          operand -> row form, DVE 32x32 transpose back to a column) for sseg
          & wseg, den = wseg + e^{sseg + L*b2 - ln 1e8}, rcol = 1/den.
          Entirely on-chip: no DRAM round-trip, no DMA on the critical chain.
          Scores flush in chunks A (tiles 0..F-4) / B (last 3) so only a short
          B-chain sits on the window boundary.
  pass 2 (node-major, bf16): u[seg,:] = (S^T * w)^T @ x via PE accumulation,
          out = u * rcol on the psum->sbuf copy

x is uploaded twice (node-major bf16 + feature-major fp8, partition-major
packed so every DMA reads contiguous bytes per partition).  Groups pipeline
lag-1: pass2(g-1) interleaves into pass1(g); score rows transpose in two
chunks (tiles 0..7 / 8..F-1) so most of the middle runs under pass-1's tail.
DMA: all bulk x on the sync HWDGE ring in exact consumption order (the ring
is FIFO; >=1MB batches), st/consts on gpsimd SWDGE, outputs on sync.
"""

import functools
import math
import sys

sys.path.insert(0, "/opt/trn_rl_repo")

import ml_dtypes
import numpy as np

import concourse.bass as bass
import concourse.tile as tile
from concourse import bacc, mybir
from concourse.bass_utils import run_bass_kernel_spmd

NCORES = 8
NSEG = 256
HID = 512
H2 = 256  # hidden//2
SEG_PER_CORE = NSEG // NCORES  # 32

BF16 = mybir.dt.bfloat16
F32 = mybir.dt.float32
E4M3 = mybir.dt.float8e4
NPBF16 = ml_dtypes.bfloat16
NPE4 = ml_dtypes.float8_e4m3
W1SCALE = 64.0  # W1/W2 pre-scaled into fp8-normal range; undone in tanh/exp
LN1E8 = math.log(1e8)

G = 4  # groups per core


def _round_up(v, m):
    return (v + m - 1) // m * m


@functools.lru_cache(maxsize=4)
def _build_graph(g_groups, gmax, spg, b2val):
    C = gmax // 128  # node chunks per group
    F = gmax // 512  # pass-1 free-dim tiles per group
    FA = F - 3  # score rows in the early chunk (small B chunk -> short boundary chain)
    FB = F - FA
    assert 9 <= F <= 16
    NB8 = C // 8
    REM = C - NB8 * 8
    p2_sizes = [8] * NB8 + ([REM] if REM else [])
    NBAT = len(p2_sizes)

    nc = bacc.Bacc(None, target_bir_lowering=False, debug=False)
    # partition-major packed: every DMA reads contiguous bytes per partition
    x_nm = nc.declare_dram_parameter("x_nm", [g_groups, 128, C, HID], BF16, isOutput=False)
    x_fm = nc.declare_dram_parameter("x_fm", [g_groups, 128, F, 4, 512], E4M3, isOutput=False)
    stc_d = nc.declare_dram_parameter("st_cs", [g_groups, 128, C, spg], BF16, isOutput=False)
    sts_d = nc.declare_dram_parameter("st_sc", [g_groups, 128, spg, C], BF16, isOutput=False)
    w1_d = nc.declare_dram_parameter("w1", [HID, H2], E4M3, isOutput=False)
    b1_d = nc.declare_dram_parameter("b1", [H2], F32, isOutput=False)
    w2v_d = nc.declare_dram_parameter("w2v", [H2, 16, 16], E4M3, isOutput=False)
    lb2_d = nc.declare_dram_parameter("lb2", [spg, g_groups], F32, isOutput=False)
    id_d = nc.declare_dram_parameter("ident", [16, 16], F32, isOutput=False)
    out_d = nc.declare_dram_parameter("out", [g_groups * spg, HID], F32, isOutput=True)

    Tanh = mybir.ActivationFunctionType.Tanh
    Exp = mybir.ActivationFunctionType.Exp
    Copy = mybir.ActivationFunctionType.Copy
    DR = mybir.MatmulPerfMode.DoubleRow

    with tile.TileContext(nc) as tc:
        with (
            tc.tile_pool(name="consts", bufs=1) as consts,
            tc.tile_pool(name="xtp", bufs=6) as xtp,
            tc.tile_pool(name="xtp1", bufs=4) as xtp1,
            tc.tile_pool(name="ttp", bufs=3) as ttp,
            tc.tile_pool(name="stp", bufs=2) as stp,
            tc.tile_pool(name="mid", bufs=2) as mid,
            tc.tile_pool(name="p2", bufs=12) as p2,
            tc.tile_pool(name="p2r", bufs=2) as p2r,
            tc.tile_pool(name="outp2", bufs=2) as outp2,
            tc.tile_pool(name="psum_h1", bufs=3, space="PSUM") as psum_h1,
            tc.tile_pool(name="psum_scp", bufs=2, space="PSUM") as psum_scp,
            tc.tile_pool(name="psum_tr", bufs=1, space="PSUM") as psum_tr,
            tc.tile_pool(name="psum_out", bufs=1, space="PSUM") as psum_out,
            tc.tile_pool(name="psum_cs", bufs=1, space="PSUM") as psum_cs,
        ):
            # -------- constants (sync ring, ahead of the bulk x stream) --------
            w1_sb = consts.tile([128, 4, H2], E4M3)  # [p, kchunk, j]
            nc.sync.dma_start(out=w1_sb, in_=w1_d[:].rearrange("(k p) j -> p k j", p=128))
            b1_sb = consts.tile([128, 2], F32)  # [p, jchunk]
            nc.sync.dma_start(out=b1_sb, in_=b1_d[:].rearrange("(j p) -> p j", p=128))
            # W2 column-variants: variant v holds 64*W2 in column v (16 cols,
            # dual-fp8 pair-dim stride 16*16=256 is a multiple of 16)
            w2v_sb = consts.tile([128, 2, 16, 16], E4M3)
            nc.sync.dma_start(
                out=w2v_sb, in_=w2v_d[:].rearrange("(j p) v o -> p j v o", p=128)
            )
            lb2_sb = consts.tile([spg, g_groups], F32)
            nc.sync.dma_start(out=lb2_sb, in_=lb2_d[:])
            id_sb = consts.tile([16, 16], F32)
            nc.sync.dma_start(out=id_sb, in_=id_d[:])
            ones_sb = consts.tile([128, 1], F32)
            nc.vector.memset(ones_sb, 1.0)

            # engine pre-touches + PE warm-up burst (HAM): dummy matmuls on
            # memset tiles run before the first x tile lands (no DMA deps).
            dumw = consts.tile([128, 128], E4M3)
            nc.vector.memset(dumw, 0.0)
            dumr = consts.tile([128, 512], E4M3)
            nc.vector.memset(dumr, 0.0)
            dum_act = consts.tile([128, 1], F32)
            nc.scalar.activation(dum_act, ones_sb, Copy)
            for _ in range(6):
                dps = psum_h1.tile([128, 512], F32, tag="h1")
                nc.tensor.matmul(dps, lhsT=dumw, rhs=dumr, start=True, stop=True)

            # ---------------- pass-1 input prefetch (sync ring) ----------------
            # small leading batches so the first tiles land early
            XFMB = [(0, 1), (1, 3), (3, 5), (5, 9), (9, F)]
            xfm_tiles = {}

            def xfm_load(g, b, eng=None):
                lo, hi = XFMB[b]
                n = hi - lo
                pool, tag = {1: (xtp1, "xtb1"), 2: (xtp1, "xtb2"),
                             4: (xtp, "xtb4")}[n]
                xt = pool.tile([128, n, 4, 512], E4M3, tag=tag, name=tag + "_t")
                (eng or nc.sync).dma_start(out=xt, in_=x_fm[g, :, lo:hi])
                xfm_tiles[(g, b)] = (xt, lo, hi)

            def xfm_tile(g, f):
                b = next(i for i, (lo, hi) in enumerate(XFMB) if lo <= f < hi)
                xt, lo, hi = xfm_tiles[(g, b)]
                if f == hi - 1:
                    xfm_tiles.pop((g, b), None)  # last use; let pool recycle
                return xt[:, f - lo]

            def st_load(g):
                s = S[g]
                s["st_cs"] = stp.tile([128, C, spg], BF16, tag="st_cs", name="stcs_t")
                nc.gpsimd.dma_start(out=s["st_cs"], in_=stc_d[g])
                s["st_sc"] = stp.tile([128, spg, C], BF16, tag="st_sc", name="stsc_t")
                nc.gpsimd.dma_start(out=s["st_sc"], in_=sts_d[g])

            # ---------------- per-group state ----------------
            S = [dict() for _ in range(g_groups)]
            pending = []  # (g, f, tt) whose score matmul is pending

            def emit_scores(g, f, tt):
                """Score matmul for tile f, emitted one iteration late so the
                in-order PE never stalls on the tanh.  Variant matmul
                accumulates tile f's score row into a psum row.  The last
                group splits chunks A (rows 0..FA-1) / B (rows 0..FB-1) so
                most of its middle overlaps pass-1; other groups use a single
                bank flushed at the window boundary (pass-2 starts a half
                window later, so the latency is free and 4 PE transposes are
                saved)."""
                s = S[g]
                if f >= FA:
                    if f == FA:
                        s["scpB"] = psum_scp.tile([16, 512], F32, tag="scp", name="scpB_t")
                    nc.tensor.matmul(
                        s["scpB"], lhsT=w2v_sb[:, :, f - FA, :], rhs=tt,
                        start=(f == FA), stop=(f == F - 1), perf_mode=DR,
                    )
                else:
                    if f == 0:
                        s["scpA"] = psum_scp.tile([16, 512], F32, tag="scp", name="scpA_t")
                    nc.tensor.matmul(
                        s["scpA"], lhsT=w2v_sb[:, :, f, :], rhs=tt,
                        start=(f == 0), stop=(f == FA - 1), perf_mode=DR,
                    )

            def emit_p1_tile(g, f):
                xt = xfm_tile(g, f)  # [128, 4, 512] fp8
                tt = ttp.tile([128, 2, 512], E4M3, tag="tt", name="tt_t")
                for j in range(2):
                    h1 = psum_h1.tile([128, 512], F32, tag="h1")
                    for k2 in range(2):
                        nc.tensor.matmul(
                            h1,
                            lhsT=w1_sb[:, 2 * k2 : 2 * k2 + 2, j * 128 : (j + 1) * 128],
                            rhs=xt[:, 2 * k2 : 2 * k2 + 2, :],
                            start=(k2 == 0),
                            stop=(k2 == 1),
                            perf_mode=DR,
                        )
                    nc.scalar.activation(
                        tt[:, j, :], h1, Tanh, bias=b1_sb[:, j : j + 1],
                        scale=1.0 / W1SCALE,
                    )
                pending.append((g, f, tt))
                while len(pending) > 1:
                    emit_scores(*pending.pop(0))

            def flush_scores():
                while pending:
                    emit_scores(*pending.pop(0))

            # -------- middle: PE-transposed scores, no DRAM round-trip --------
            # score rows [F, 512] -> node-major sc [128, 4, F]: node chunk
            # c = j*F + f holds score-row f's column block j (x_fm permuted).
            def flush_copy(g, scpkey, srkey, rows):
                s = S[g]
                s[srkey] = mid.tile([16, 512], F32, tag=srkey, name=srkey + "_t")
                nc.vector.tensor_copy(s[srkey][0:rows], s.pop(scpkey)[0:rows])

            def flush_tr(g, srkey, flo, fhi):
                s = S[g]
                sr = s.pop(srkey)
                n = fhi - flo
                tr = psum_tr.tile([128, 4, 16], F32, tag="tr", name="tr_t")
                for j in range(4):
                    nc.tensor.transpose(
                        tr[:, j, 0:n], sr[0:n, 128 * j : 128 * (j + 1)],
                        id_sb[0:n, 0:n],
                    )
                if flo == 0:
                    s["sc"] = mid.tile([128, 4, F], F32, tag="sc", name="sc_t")
                nc.vector.tensor_copy(s["sc"][:, :, flo:fhi], tr[:, :, 0:n])

            def flushA_copy(g):
                flush_copy(g, "scpA", "srA", FA)

            def flushA_tr(g):
                flush_tr(g, "srA", 0, FA)

            def seg_reduce(s, vec, col0, flo, fhi, part):
                """part[:, col0:col0+spg] = one-hot partial segment sums of
                vec[:, :, flo:fhi] over node chunks c = j*F + f."""
                nf = fhi - flo
                nc.vector.tensor_mul(
                    s["prod"][:, :, :, 0:nf],
                    s["st_sc"][:].rearrange("p s (j f) -> p s j f", j=4)[
                        :, :, :, flo:fhi
                    ],
                    vec[:, :, flo:fhi]
                    .rearrange("p (x j) f -> p x j f", x=1)
                    .to_broadcast([128, spg, 4, nf]),
                )
                nc.vector.reduce_sum(
                    out=part[:, col0 : col0 + spg],
                    in_=s["prod"][:, :, :, 0:nf],
                    axis=mybir.AxisListType.XY,
                )

            def mid_part(g, flo, fhi, partkey):
                """Exp + swt + partial reduces over score-row range [flo, fhi)
                (swt first: it gates pass-2)."""
                s = S[g]
                n = fhi - flo
                if flo == 0:
                    s["w"] = mid.tile([128, 4, F], F32, tag="w", name="w_t")
                nc.scalar.activation(
                    s["w"][:, :, flo:fhi], s["sc"][:, :, flo:fhi], Exp, bias=b2val,
                    scale=1.0 / W1SCALE,
                )
                if flo == 0:
                    s["swt"] = mid.tile([128, C, spg], BF16, tag="swt", name="swt_t")
                nc.vector.tensor_mul(
                    s["swt"][:].rearrange("p (j f) s -> p j f s", j=4)[
                        :, :, flo:fhi, :
                    ],
                    s["st_cs"][:].rearrange("p (j f) s -> p j f s", j=4)[
                        :, :, flo:fhi, :
                    ],
                    s["w"][:, :, flo:fhi].to_broadcast([128, 4, n, spg]),
                )
                if "prod" not in s:
                    s["prod"] = mid.tile([128, spg, 4, F], F32, tag="prod", name="prod_t")
                s[partkey] = mid.tile([128, 40], F32, tag=partkey, name=partkey + "_t")
                seg_reduce(s, s["sc"], 0, flo, fhi, s[partkey])
                seg_reduce(s, s["w"], 32, flo, fhi, s[partkey])
                if fhi == F:
                    s.pop("prod")
                    s.pop("sc")

            def midA(g):
                mid_part(g, 0, FA, "partA")

            def midB(g):
                mid_part(g, FA, F, "partB")

            def mid_colsum(g):
                """PE cross-partition sum as a ROW [1, 40] (lhsT = ones: cheap
                fp32 ldweights; fp32 rhs is only 40 elements)."""
                s = S[g]
                cs = psum_cs.tile([1, 40], F32, tag="cs", name="cs_t")
                two = "partB" in s
                nc.tensor.matmul(cs, lhsT=ones_sb, rhs=s.pop("partA"),
                                 start=True, stop=not two)
                if two:
                    nc.tensor.matmul(cs, lhsT=ones_sb, rhs=s.pop("partB"),
                                     start=False, stop=True)
                s["cs"] = cs

            def mid_den(g):
                """Row->column via DVE 32x32 transposes, then the den chain."""
                s = S[g]
                rowt = mid.tile([32, 64], F32, tag="rowt")
                nc.vector.tensor_copy(rowt[0:1, 0:40], s.pop("cs"))
                colt = mid.tile([32, 64], F32, tag="colt")
                nc.vector.transpose(colt[:, 0:32], rowt[:, 0:32])
                nc.vector.transpose(colt[:, 32:64], rowt[:, 32:64])
                esseg = mid.tile([spg, 1], F32, tag="esseg")
                nc.scalar.activation(
                    esseg, colt[0:spg, 0:1], Exp, bias=lb2_sb[0:spg, g : g + 1],
                    scale=1.0 / W1SCALE,
                )
                den = mid.tile([spg, 1], F32, tag="den")
                nc.vector.tensor_add(den, esseg, colt[0:spg, 32:33])
                s["rcol"] = mid.tile([spg, 1], F32, tag="rcol", name="rcol_t")
                nc.vector.reciprocal(s["rcol"], den)

            # ---------------- pass 2 ----------------
            def xnm_load(g, k):
                s = S[g]
                c0 = 8 * k
                nb = p2_sizes[k]
                tag = "xnm8" if nb == 8 else "xnm4"
                pool = p2 if nb == 8 else p2r
                xt2 = pool.tile([128, nb, 512], BF16, tag=tag, name=tag + "_t")
                nc.sync.dma_start(out=xt2, in_=x_nm[g][:, c0 : c0 + nb, :])
                s.setdefault("xt2", {})[k] = xt2

            def p2_batch(g, k):
                s = S[g]
                if k == 0:
                    s["outp"] = psum_out.tile([spg, 512], F32, tag="outp", name="outp_t")
                xt2 = s["xt2"].pop(k)
                c0 = 8 * k
                for i in range(p2_sizes[k]):
                    c = c0 + i
                    nc.tensor.matmul(
                        s["outp"], lhsT=s["swt"][:, c, :], rhs=xt2[:, i, :],
                        start=(c == 0), stop=(c == C - 1),
                    )

            def p2_finish(g):
                s = S[g]
                out_sb = outp2.tile([spg, HID], F32, tag="out_sb")
                nc.vector.tensor_mul(
                    out_sb, s.pop("outp"), s.pop("rcol").to_broadcast([spg, HID])
                )
                nc.sync.dma_start(out=out_d[g * spg : (g + 1) * spg, :], in_=out_sb)
                s.clear()

            # ---------------- schedule ----------------
            # startup: group 0's first batches on the (otherwise idle) scalar
            # HWDGE ring so they overlap the consts on the sync ring
            xfm_load(0, 0, eng=nc.scalar)
            xfm_load(0, 1, eng=nc.scalar)
            xfm_load(0, 2, eng=nc.scalar)
            xfm_load(0, 3)
            xfm_load(0, 4)
            st_load(0)

            # window g: pass-1(g) + pass2(g-1) batches + prefetch.  The sync
            # ring is FIFO: issue order == consumption order (xfm(g+1) first,
            # then xnm(g-1) tail + xnm(g) head).
            P2SLOT = {5: 0, 6: 1, 7: 2, 8: 3, 10: 4, 12: 5}
            XNMSLOT = {9: 0, 10: 1, 11: 2, 12: 3}  # xnm(g) head, in window g
            # last group: no xfm(g+1) stream, so its xnm loads all fit early
            XNMLAST = {3: 0, 4: 1, 5: 2, 6: 3, 9: 4, 10: 5, 11: 6}
            XNMCARRY = {0: 4, 1: 5, 2: 6}  # xnm(g-1) tail, in window g
            for g in range(g_groups):
                last = g == g_groups - 1
                for f in range(F):
                    emit_p1_tile(g, f)
                    if not last and 2 <= f < 2 + len(XFMB):
                        xfm_load(g + 1, f - 2)
                    if not last and f == 7:
                        st_load(g + 1)
                    if g >= 1:
                        k = XNMCARRY.get(f)
                        if k is not None and k < NBAT:
                            xnm_load(g - 1, k)
                    k = (XNMLAST if last else XNMSLOT).get(f)
                    if k is not None and k < NBAT:
                        xnm_load(g, k)
                    if f == FA:
                        flushA_copy(g)
                    elif f == FA + 1:
                        flushA_tr(g)
                    elif f == FA + 2:
                        midA(g)
                    if g >= 1:
                        k = P2SLOT.get(f)
                        if k is not None and k < NBAT:
                            p2_batch(g - 1, k)
                        if f == 9:
                            mid_colsum(g - 1)
                        elif f == 11:
                            mid_den(g - 1)
                # inter-window: late score chunk + last pass-2 batch of g-1
                flush_scores()
                flush_copy(g, "scpB", "srB", FB)
                if g >= 1 and NBAT > 6:
                    p2_batch(g - 1, 6)
                flush_tr(g, "srB", FA, F)
                midB(g)
                if g >= 1:
                    p2_finish(g - 1)
            # tail: last group's middle + pass-2 (xnm(gl) fully loaded in-window)
            gl = g_groups - 1
            p2_batch(gl, 0)
            p2_batch(gl, 1)
            mid_colsum(gl)
            mid_den(gl)
            for k in range(2, NBAT):
                p2_batch(gl, k)
            p2_finish(gl)

    nc.compile()
    return nc


def _prepare(x, batch, W1, b1, W2, b2, g_groups):
    """Host-side sharding/packing.  Returns (in_maps, gmax, spg, b2val)."""
    x = np.ascontiguousarray(np.asarray(x, dtype=np.float32))
    batch = np.asarray(batch).astype(np.int64)
    spg = SEG_PER_CORE // g_groups

    bounds = np.searchsorted(batch, np.arange(NSEG + 1))
    glens = bounds[spg:NSEG + 1:spg] - bounds[0:NSEG:spg]  # len per (core,group)
    gmax = max(4608, _round_up(int(glens.max()), 512))
    C = gmax // 128
    F = gmax // 512

    xb = x.astype(NPBF16)
    x8 = x.astype(NPE4)
    w1b = np.ascontiguousarray((np.asarray(W1, np.float32) * W1SCALE).astype(NPE4))
    w2s = (np.asarray(W2, np.float32) * W1SCALE).astype(NPE4).reshape(H2)
    w2v = np.zeros((H2, 16, 16), NPE4)
    for v in range(16):
        w2v[:, v, v] = w2s
    w2v = np.ascontiguousarray(w2v)
    b1f = np.ascontiguousarray(np.asarray(b1, np.float32).reshape(H2))
    b2val = float(np.asarray(b2, np.float32).reshape(-1)[0])
    ident = np.eye(16, dtype=np.float32)

    # feature-major slot i = f*512 + j*128 + p holds node (j*F + f)*128 + p:
    # the PE block-transpose of score row f's column block j then lands
    # node-major (node chunk c = j*F + f)
    ii = np.arange(gmax)
    fi, ji, pi = ii // 512, (ii % 512) // 128, ii % 128
    perm = (ji * F + fi) * 128 + pi

    in_maps = []
    for core in range(NCORES):
        x_nm = np.zeros((g_groups, 128, C, HID), NPBF16)
        x_fm = np.zeros((g_groups, 128, F, 4, 512), NPE4)
        st_cs = np.zeros((g_groups, 128, C, spg), NPBF16)
        lb2 = np.zeros((spg, g_groups), np.float32)
        for g in range(g_groups):
            s0 = core * SEG_PER_CORE + g * spg
            n0, n1 = int(bounds[s0]), int(bounds[s0 + spg])
            L = n1 - n0
            xg = np.zeros((gmax, HID), NPBF16)
            xg[:L] = xb[n0:n1]
            # node-major: [p, c, hid], node = c*128 + p
            x_nm[g] = xg.reshape(C, 128, HID).transpose(1, 0, 2)
            xg8 = np.zeros((gmax, HID), NPE4)
            xg8[:L] = x8[n0:n1]
            xT = np.ascontiguousarray(xg8[perm].T)  # [HID, gmax]
            x_fm[g] = xT.reshape(4, 128, F, 512).transpose(1, 2, 0, 3)
            oh = np.zeros((gmax, spg), np.float32)
            oh[np.arange(L), (batch[n0:n1] - s0).astype(np.int64)] = 1.0
            st_cs[g] = oh.reshape(C, 128, spg).transpose(1, 0, 2)
            segl = bounds[s0 + 1 : s0 + spg + 1] - bounds[s0:s0 + spg]
            lb2[:, g] = segl.astype(np.float32) * b2val - LN1E8
        st_sc = np.ascontiguousarray(st_cs.transpose(0, 1, 3, 2))
        in_maps.append(
            {
                "x_nm": x_nm,
                "x_fm": x_fm,
                "st_cs": np.ascontiguousarray(st_cs),
                "st_sc": st_sc,
                "w1": w1b,
                "b1": b1f,
                "w2v": w2v,
                "lb2": np.ascontiguousarray(lb2),
                "ident": ident,
            }
        )
    return in_maps, gmax, spg, b2val


def _run(inputs, trace=False, **run_kwargs):
    in_maps, gmax, spg, b2val = _prepare(
        inputs["x"], inputs["batch"], inputs["W1"], inputs["b1"],
        inputs["W2"], inputs["b2"], G,
    )
    nc = _build_graph(G, gmax, spg, b2val)
    res = run_bass_kernel_spmd(
        nc, in_maps, core_ids=list(range(NCORES)), trace=trace, **run_kwargs
    )
    out = np.concatenate([r["out"] for r in res.results], axis=0)
    return out.astype(np.float32), res


def kernel(**inputs) -> np.ndarray:
    out, _ = _run(inputs, trace=False)
    return out


# revision 53
# speedup vs baseline: 1.0065x; 1.0065x over previous
"""AttentionPooling (segment softmax-pool) Trainium2 kernel, 8 NeuronCores.

Sharding: each core owns 32 consecutive segments (contiguous node range since
`batch` is sorted); the 32 segments split into G=4 groups of spg=8 segments
whose node ranges are padded to a fixed gmax so all cores/groups run one
static graph.  All segment reductions are core-local; no collectives.

Math: the reference computes att = exp(s - mx)/(seg(exp(s - mx)) + 1e-8)
with mx = seg(s) (a scatter-add "max" stabilizer).  We use the exact
factored form
    out[seg] = (sum_n e^{s_n} x_n) / (sum_n e^{s_n} + 1e-8 * e^{mx_seg})
so pass-2 runs on unnormalized weights w = e^s and the 1/denominator is a
per-segment scale folded into the final [spg, 512] psum->sbuf copy.  No
per-node max gather, no per-node normalize.

Per-core, per-group pipeline:
  pass 1 (feature-major, fp8 e4m3 DoubleRow / fp32 accum):
          h = tanh(W1^T @ x^T) ; scores via W2 "column-variant" matmuls that
          accumulate tile f's score row into PSUM ROW f of a per-group score
          bank (weights pre-scaled x64 into fp8-normal range, undone later)
  middle: score bank -> SBUF -> PE-transposed ([FA,128] blocks -> [128,FA])
          into node-major [128, 4, F] (the x_fm node permutation makes chunk
          c = j*F + f land exactly there, j = tile-column block); w = exp(s),
          one-hot partial reduce (DVE) + PE column-sum (ones as the stationary
          operand -> row form, DVE 32x32 transpose back to a column) for sseg
          & wseg, den = wseg + e^{sseg + L*b2 - ln 1e8}, rcol = 1/den.
          Entirely on-chip: no DRAM round-trip, no DMA on the critical chain.
          Scores flush in chunks A (tiles 0..F-4) / B (last 3) so only a short
          B-chain sits on the window boundary.
  pass 2 (node-major, bf16): u[seg,:] = (S^T * w)^T @ x via PE accumulation,
          out = u * rcol on the psum->sbuf copy

x is uploaded twice (node-major bf16 + feature-major fp8, partition-major
packed so every DMA reads contiguous bytes per partition).  Groups pipeline
lag-1: pass2(g-1) interleaves into pass1(g); score rows transpose in two
chunks (tiles 0..7 / 8..F-1) so most of the middle runs under pass-1's tail.
DMA: all bulk x on the sync HWDGE ring in exact consumption order (the ring
is FIFO; >=1MB batches), st/consts on gpsimd SWDGE, outputs on sync.
"""

import functools
import math
import sys

sys.path.insert(0, "/opt/trn_rl_repo")

import ml_dtypes
import numpy as np

import concourse.bass as bass
import concourse.tile as tile
from concourse import bacc, mybir
from concourse.bass_utils import run_bass_kernel_spmd

NCORES = 8
NSEG = 256
HID = 512
H2 = 256  # hidden//2
SEG_PER_CORE = NSEG // NCORES  # 32

BF16 = mybir.dt.bfloat16
F32 = mybir.dt.float32
E4M3 = mybir.dt.float8e4
NPBF16 = ml_dtypes.bfloat16
NPE4 = ml_dtypes.float8_e4m3
W1SCALE = 64.0  # W1/W2 pre-scaled into fp8-normal range; undone in tanh/exp
LN1E8 = math.log(1e8)

G = 4  # groups per core


def _round_up(v, m):
    return (v + m - 1) // m * m


@functools.lru_cache(maxsize=4)
def _build_graph(g_groups, gmax, spg, b2val):
    C = gmax // 128  # node chunks per group
    F = gmax // 512  # pass-1 free-dim tiles per group
    FA = F - 3  # score rows in the early chunk (small B chunk -> short boundary chain)
    FB = F - FA
    assert 9 <= F <= 16
    NB8 = C // 8
    REM = C - NB8 * 8
    p2_sizes = [8] * NB8 + ([REM] if REM else [])
    NBAT = len(p2_sizes)

    nc = bacc.Bacc(None, target_bir_lowering=False, debug=False)
    # partition-major packed: every DMA reads contiguous bytes per partition
    x_nm = nc.declare_dram_parameter("x_nm", [g_groups, 128, C, HID], BF16, isOutput=False)
    x_fm = nc.declare_dram_parameter("x_fm", [g_groups, 128, F, 4, 512], E4M3, isOutput=False)
    stc_d = nc.declare_dram_parameter("st_cs", [g_groups, 128, C, spg], BF16, isOutput=False)
    sts_d = nc.declare_dram_parameter("st_sc", [g_groups, 128, spg, C], BF16, isOutput=False)
    w1_d = nc.declare_dram_parameter("w1", [HID, H2], E4M3, isOutput=False)
    b1_d = nc.declare_dram_parameter("b1", [H2], F32, isOutput=False)
    w2v_d = nc.declare_dram_parameter("w2v", [H2, 16, 16], E4M3, isOutput=False)
    lb2_d = nc.declare_dram_parameter("lb2", [spg, g_groups], F32, isOutput=False)
    id_d = nc.declare_dram_parameter("ident", [16, 16], F32, isOutput=False)
    out_d = nc.declare_dram_parameter("out", [g_groups * spg, HID], F32, isOutput=True)

    Tanh = mybir.ActivationFunctionType.Tanh
    Exp = mybir.ActivationFunctionType.Exp
    Copy = mybir.ActivationFunctionType.Copy
    DR = mybir.MatmulPerfMode.DoubleRow

    with tile.TileContext(nc) as tc:
        with (
            tc.tile_pool(name="consts", bufs=1) as consts,
            tc.tile_pool(name="xtp", bufs=6) as xtp,
            tc.tile_pool(name="xtp1", bufs=4) as xtp1,
            tc.tile_pool(name="ttp", bufs=3) as ttp,
            tc.tile_pool(name="stp", bufs=2) as stp,
            tc.tile_pool(name="mid", bufs=2) as mid,
            tc.tile_pool(name="p2", bufs=12) as p2,
            tc.tile_pool(name="p2r", bufs=2) as p2r,
            tc.tile_pool(name="outp2", bufs=2) as outp2,
            tc.tile_pool(name="psum_h1", bufs=3, space="PSUM") as psum_h1,
            tc.tile_pool(name="psum_scp", bufs=2, space="PSUM") as psum_scp,
            tc.tile_pool(name="psum_tr", bufs=1, space="PSUM") as psum_tr,
            tc.tile_pool(name="psum_out", bufs=1, space="PSUM") as psum_out,
            tc.tile_pool(name="psum_cs", bufs=1, space="PSUM") as psum_cs,
        ):
            # -------- constants (sync ring, ahead of the bulk x stream) --------
            w1_sb = consts.tile([128, 4, H2], E4M3)  # [p, kchunk, j]
            nc.sync.dma_start(out=w1_sb, in_=w1_d[:].rearrange("(k p) j -> p k j", p=128))
            b1_sb = consts.tile([128, 2], F32)  # [p, jchunk]
            nc.sync.dma_start(out=b1_sb, in_=b1_d[:].rearrange("(j p) -> p j", p=128))
            # W2 column-variants: variant v holds 64*W2 in column v (16 cols,
            # dual-fp8 pair-dim stride 16*16=256 is a multiple of 16)
            w2v_sb = consts.tile([128, 2, 16, 16], E4M3)
            nc.sync.dma_start(
                out=w2v_sb, in_=w2v_d[:].rearrange("(j p) v o -> p j v o", p=128)
            )
            lb2_sb = consts.tile([spg, g_groups], F32)
            nc.sync.dma_start(out=lb2_sb, in_=lb2_d[:])
            id_sb = consts.tile([16, 16], F32)
            nc.sync.dma_start(out=id_sb, in_=id_d[:])
            ones_sb = consts.tile([128, 1], F32)
            nc.vector.memset(ones_sb, 1.0)

            # engine pre-touches + PE warm-up burst (HAM): dummy matmuls on
            # memset tiles run before the first x tile lands (no DMA deps).
            dumw = consts.tile([128, 128], E4M3)
            nc.vector.memset(dumw, 0.0)
            dumr = consts.tile([128, 512], E4M3)
            nc.vector.memset(dumr, 0.0)
            dum_act = consts.tile([128, 1], F32)
            nc.scalar.activation(dum_act, ones_sb, Copy)
            for _ in range(6):
                dps = psum_h1.tile([128, 512], F32, tag="h1")
                nc.tensor.matmul(dps, lhsT=dumw, rhs=dumr, start=True, stop=True)

            # ---------------- pass-1 input prefetch (sync ring) ----------------
            # small leading batches so the first tiles land early
            XFMB = [(0, 1), (1, 3), (3, 5), (5, 9), (9, F)]
            xfm_tiles = {}

            def xfm_load(g, b, eng=None):
                lo, hi = XFMB[b]
                n = hi - lo
                pool, tag = {1: (xtp1, "xtb1"), 2: (xtp1, "xtb2"),
                             4: (xtp, "xtb4")}[n]
                xt = pool.tile([128, n, 4, 512], E4M3, tag=tag, name=tag + "_t")
                (eng or nc.sync).dma_start(out=xt, in_=x_fm[g, :, lo:hi])
                xfm_tiles[(g, b)] = (xt, lo, hi)

            def xfm_tile(g, f):
                b = next(i for i, (lo, hi) in enumerate(XFMB) if lo <= f < hi)
                xt, lo, hi = xfm_tiles[(g, b)]
                if f == hi - 1:
                    xfm_tiles.pop((g, b), None)  # last use; let pool recycle
                return xt[:, f - lo]

            def st_load(g):
                s = S[g]
                s["st_cs"] = stp.tile([128, C, spg], BF16, tag="st_cs", name="stcs_t")
                nc.gpsimd.dma_start(out=s["st_cs"], in_=stc_d[g])
                s["st_sc"] = stp.tile([128, spg, C], BF16, tag="st_sc", name="stsc_t")
                nc.gpsimd.dma_start(out=s["st_sc"], in_=sts_d[g])

            # ---------------- per-group state ----------------
            S = [dict() for _ in range(g_groups)]
            pending = []  # (g, f, tt) whose score matmul is pending

            def emit_scores(g, f, tt):
                """Score matmul for tile f, emitted one iteration late so the
                in-order PE never stalls on the tanh.  Variant matmul
                accumulates tile f's score row into a psum row.  The last
                group splits chunks A (rows 0..FA-1) / B (rows 0..FB-1) so
                most of its middle overlaps pass-1; other groups use a single
                bank flushed at the window boundary (pass-2 starts a half
                window later, so the latency is free and 4 PE transposes are
                saved)."""
                s = S[g]
                if f >= FA:
                    if f == FA:
                        s["scpB"] = psum_scp.tile([16, 512], F32, tag="scp", name="scpB_t")
                    nc.tensor.matmul(
                        s["scpB"], lhsT=w2v_sb[:, :, f - FA, :], rhs=tt,
                        start=(f == FA), stop=(f == F - 1), perf_mode=DR,
                    )
                else:
                    if f == 0:
                        s["scpA"] = psum_scp.tile([16, 512], F32, tag="scp", name="scpA_t")
                    nc.tensor.matmul(
                        s["scpA"], lhsT=w2v_sb[:, :, f, :], rhs=tt,
                        start=(f == 0), stop=(f == FA - 1), perf_mode=DR,
                    )

            def emit_p1_tile(g, f):
                xt = xfm_tile(g, f)  # [128, 4, 512] fp8
                tt = ttp.tile([128, 2, 512], E4M3, tag="tt", name="tt_t")
                for j in range(2):
                    h1 = psum_h1.tile([128, 512], F32, tag="h1")
                    for k2 in range(2):
                        nc.tensor.matmul(
                            h1,
                            lhsT=w1_sb[:, 2 * k2 : 2 * k2 + 2, j * 128 : (j + 1) * 128],
                            rhs=xt[:, 2 * k2 : 2 * k2 + 2, :],
                            start=(k2 == 0),
                            stop=(k2 == 1),
                            perf_mode=DR,
                        )
                    nc.scalar.activation(
                        tt[:, j, :], h1, Tanh, bias=b1_sb[:, j : j + 1],
                        scale=1.0 / W1SCALE,
                    )
                pending.append((g, f, tt))
                while len(pending) > 1:
                    emit_scores(*pending.pop(0))

            def flush_scores():
                while pending:
                    emit_scores(*pending.pop(0))

            # -------- middle: PE-transposed scores, no DRAM round-trip --------
            # score rows [F, 512] -> node-major sc [128, 4, F]: node chunk
            # c = j*F + f holds score-row f's column block j (x_fm permuted).
            def flush_copy(g, scpkey, srkey, rows):
                s = S[g]
                s[srkey] = mid.tile([16, 512], F32, tag=srkey, name=srkey + "_t")
                nc.vector.tensor_copy(s[srkey][0:rows], s.pop(scpkey)[0:rows])

            def flush_tr(g, srkey, flo, fhi):
                s = S[g]
                sr = s.pop(srkey)
                n = fhi - flo
                tr = psum_tr.tile([128, 4, 16], F32, tag="tr", name="tr_t")
                for j in range(4):
                    nc.tensor.transpose(
                        tr[:, j, 0:n], sr[0:n, 128 * j : 128 * (j + 1)],
                        id_sb[0:n, 0:n],
                    )
                if flo == 0:
                    s["sc"] = mid.tile([128, 4, F], F32, tag="sc", name="sc_t")
                nc.vector.tensor_copy(s["sc"][:, :, flo:fhi], tr[:, :, 0:n])

            def flushA_copy(g):
                flush_copy(g, "scpA", "srA", FA)

            def flushA_tr(g):
                flush_tr(g, "srA", 0, FA)

            def seg_reduce(s, vec, col0, flo, fhi, part):
                """part[:, col0:col0+spg] = one-hot partial segment sums of
                vec[:, :, flo:fhi] over node chunks c = j*F + f."""
                nf = fhi - flo
                nc.vector.tensor_mul(
                    s["prod"][:, :, :, 0:nf],
                    s["st_sc"][:].rearrange("p s (j f) -> p s j f", j=4)[
                        :, :, :, flo:fhi
                    ],
                    vec[:, :, flo:fhi]
                    .rearrange("p (x j) f -> p x j f", x=1)
                    .to_broadcast([128, spg, 4, nf]),
                )
                nc.vector.reduce_sum(
                    out=part[:, col0 : col0 + spg],
                    in_=s["prod"][:, :, :, 0:nf],
                    axis=mybir.AxisListType.XY,
                )

            def mid_part(g, flo, fhi, partkey):
                """Exp + swt + partial reduces over score-row range [flo, fhi)
                (swt first: it gates pass-2)."""
                s = S[g]
                n = fhi - flo
                if flo == 0:
                    s["w"] = mid.tile([128, 4, F], F32, tag="w", name="w_t")
                nc.scalar.activation(
                    s["w"][:, :, flo:fhi], s["sc"][:, :, flo:fhi], Exp, bias=b2val,
                    scale=1.0 / W1SCALE,
                )
                if flo == 0:
                    s["swt"] = mid.tile([128, C, spg], BF16, tag="swt", name="swt_t")
                nc.vector.tensor_mul(
                    s["swt"][:].rearrange("p (j f) s -> p j f s", j=4)[
                        :, :, flo:fhi, :
                    ],
                    s["st_cs"][:].rearrange("p (j f) s -> p j f s", j=4)[
                        :, :, flo:fhi, :
                    ],
                    s["w"][:, :, flo:fhi].to_broadcast([128, 4, n, spg]),
                )
                if "prod" not in s:
                    s["prod"] = mid.tile([128, spg, 4, F], F32, tag="prod", name="prod_t")
                s[partkey] = mid.tile([128, 40], F32, tag=partkey, name=partkey + "_t")
                seg_reduce(s, s["sc"], 0, flo, fhi, s[partkey])
                seg_reduce(s, s["w"], 32, flo, fhi, s[partkey])
                if fhi == F:
                    s.pop("prod")
                    s.pop("sc")

            def midA(g):
                mid_part(g, 0, FA, "partA")

            def midB(g):
                mid_part(g, FA, F, "partB")

            def mid_colsum(g):
                """PE cross-partition sum as a ROW [1, 40] (lhsT = ones: cheap
                fp32 ldweights; fp32 rhs is only 40 elements)."""
                s = S[g]
                cs = psum_cs.tile([1, 40], F32, tag="cs", name="cs_t")
                two = "partB" in s
                nc.tensor.matmul(cs, lhsT=ones_sb, rhs=s.pop("partA"),
                                 start=True, stop=not two)
                if two:
                    nc.tensor.matmul(cs, lhsT=ones_sb, rhs=s.pop("partB"),
                                     start=False, stop=True)
                s["cs"] = cs

            def mid_den(g):
                """Row->column via DVE 32x32 transposes, then the den chain."""
                s = S[g]
                rowt = mid.tile([32, 64], F32, tag="rowt")
                nc.vector.tensor_copy(rowt[0:1, 0:40], s.pop("cs"))
                colt = mid.tile([32, 64], F32, tag="colt")
                nc.vector.transpose(colt[:, 0:32], rowt[:, 0:32])
                nc.vector.transpose(colt[:, 32:64], rowt[:, 32:64])
                esseg = mid.tile([spg, 1], F32, tag="esseg")
                nc.scalar.activation(
                    esseg, colt[0:spg, 0:1], Exp, bias=lb2_sb[0:spg, g : g + 1],
                    scale=1.0 / W1SCALE,
                )
                den = mid.tile([spg, 1], F32, tag="den")
                nc.vector.tensor_add(den, esseg, colt[0:spg, 32:33])
                s["rcol"] = mid.tile([spg, 1], F32, tag="rcol", name="rcol_t")
                nc.vector.reciprocal(s["rcol"], den)

            # ---------------- pass 2 ----------------
            def xnm_load(g, k):
                s = S[g]
                c0 = 8 * k
                nb = p2_sizes[k]
                tag = "xnm8" if nb == 8 else "xnm4"
                pool = p2 if nb == 8 else p2r
                xt2 = pool.tile([128, nb, 512], BF16, tag=tag, name=tag + "_t")
                nc.sync.dma_start(out=xt2, in_=x_nm[g][:, c0 : c0 + nb, :])
                s.setdefault("xt2", {})[k] = xt2

            def p2_batch(g, k):
                s = S[g]
                if k == 0:
                    s["outp"] = psum_out.tile([spg, 512], F32, tag="outp", name="outp_t")
                xt2 = s["xt2"].pop(k)
                c0 = 8 * k
                for i in range(p2_sizes[k]):
                    c = c0 + i
                    nc.tensor.matmul(
                        s["outp"], lhsT=s["swt"][:, c, :], rhs=xt2[:, i, :],
                        start=(c == 0), stop=(c == C - 1),
                    )

            def p2_finish(g):
                s = S[g]
                out_sb = outp2.tile([spg, HID], F32, tag="out_sb")
                nc.vector.tensor_mul(
                    out_sb, s.pop("outp"), s.pop("rcol").to_broadcast([spg, HID])
                )
                nc.sync.dma_start(out=out_d[g * spg : (g + 1) * spg, :], in_=out_sb)
                s.clear()

            # ---------------- schedule ----------------
            # startup: group 0's first batches on the (otherwise idle) scalar
            # HWDGE ring so they overlap the consts on the sync ring
            xfm_load(0, 0, eng=nc.scalar)
            xfm_load(0, 1, eng=nc.scalar)
            xfm_load(0, 2, eng=nc.scalar)
            xfm_load(0, 3)
            xfm_load(0, 4)
            st_load(0)

            # window g: pass-1(g) + pass2(g-1) batches + prefetch.  The sync
            # ring is FIFO: issue order == consumption order (xfm(g+1) first,
            # then xnm(g-1) tail + xnm(g) head).
            P2SLOT = {5: 0, 6: 1, 7: 2, 8: 3, 10: 4, 12: 5}
            XNMSLOT = {9: 0, 10: 1, 11: 2, 12: 3}  # xnm(g) head, in window g
            # last group: no xfm(g+1) stream, so its xnm loads all fit early
            XNMLAST = {3: 0, 4: 1, 5: 2, 6: 3, 9: 4, 10: 5, 11: 6}
            XNMCARRY = {0: 4, 1: 5, 2: 6}  # xnm(g-1) tail, in window g
            for g in range(g_groups):
                last = g == g_groups - 1
                for f in range(F):
                    emit_p1_tile(g, f)
                    if g >= 1 and f == 0:
                        # deferred B-phase of the previous group's middle: its
                        # exp now sits BEHIND this window's first tanhs in the
                        # scalar queue (no head-of-line stall on the assemble)
                        midB(g - 1)
                    if not last and 2 <= f < 2 + len(XFMB):
                        xfm_load(g + 1, f - 2)
                    if not last and f == 7:
                        st_load(g + 1)
                    if g >= 1:
                        k = XNMCARRY.get(f)
                        if k is not None and k < NBAT:
                            xnm_load(g - 1, k)
                    k = (XNMLAST if last else XNMSLOT).get(f)
                    if k is not None and k < NBAT:
                        xnm_load(g, k)
                    if f == FA:
                        flushA_copy(g)
                    elif f == FA + 1:
                        flushA_tr(g)
                    elif f == FA + 2:
                        midA(g)
                    if g >= 1:
                        k = P2SLOT.get(f)
                        if k is not None and k < NBAT:
                            p2_batch(g - 1, k)
                        if f == 9:
                            mid_colsum(g - 1)
                        elif f == 11:
                            mid_den(g - 1)
                # inter-window: late score chunk + last pass-2 batch of g-1
                flush_scores()
                flush_copy(g, "scpB", "srB", FB)
                if g >= 1 and NBAT > 6:
                    p2_batch(g - 1, 6)
                flush_tr(g, "srB", FA, F)
                if last:
                    midB(g)  # tail-critical: pass-2 starts right after
                if g >= 1:
                    p2_finish(g - 1)
            # tail: last group's middle + pass-2 (xnm(gl) fully loaded in-window)
            gl = g_groups - 1
            p2_batch(gl, 0)
            p2_batch(gl, 1)
            mid_colsum(gl)
            mid_den(gl)
            for k in range(2, NBAT):
                p2_batch(gl, k)
            p2_finish(gl)

    nc.compile()
    return nc


def _prepare(x, batch, W1, b1, W2, b2, g_groups):
    """Host-side sharding/packing.  Returns (in_maps, gmax, spg, b2val)."""
    x = np.ascontiguousarray(np.asarray(x, dtype=np.float32))
    batch = np.asarray(batch).astype(np.int64)
    spg = SEG_PER_CORE // g_groups

    bounds = np.searchsorted(batch, np.arange(NSEG + 1))
    glens = bounds[spg:NSEG + 1:spg] - bounds[0:NSEG:spg]  # len per (core,group)
    gmax = max(4608, _round_up(int(glens.max()), 512))
    C = gmax // 128
    F = gmax // 512

    xb = x.astype(NPBF16)
    x8 = x.astype(NPE4)
    w1b = np.ascontiguousarray((np.asarray(W1, np.float32) * W1SCALE).astype(NPE4))
    w2s = (np.asarray(W2, np.float32) * W1SCALE).astype(NPE4).reshape(H2)
    w2v = np.zeros((H2, 16, 16), NPE4)
    for v in range(16):
        w2v[:, v, v] = w2s
    w2v = np.ascontiguousarray(w2v)
    b1f = np.ascontiguousarray(np.asarray(b1, np.float32).reshape(H2))
    b2val = float(np.asarray(b2, np.float32).reshape(-1)[0])
    ident = np.eye(16, dtype=np.float32)

    # feature-major slot i = f*512 + j*128 + p holds node (j*F + f)*128 + p:
    # the PE block-transpose of score row f's column block j then lands
    # node-major (node chunk c = j*F + f)
    ii = np.arange(gmax)
    fi, ji, pi = ii // 512, (ii % 512) // 128, ii % 128
    perm = (ji * F + fi) * 128 + pi

    in_maps = []
    for core in range(NCORES):
        x_nm = np.zeros((g_groups, 128, C, HID), NPBF16)
        x_fm = np.zeros((g_groups, 128, F, 4, 512), NPE4)
        st_cs = np.zeros((g_groups, 128, C, spg), NPBF16)
        lb2 = np.zeros((spg, g_groups), np.float32)
        for g in range(g_groups):
            s0 = core * SEG_PER_CORE + g * spg
            n0, n1 = int(bounds[s0]), int(bounds[s0 + spg])
            L = n1 - n0
            xg = np.zeros((gmax, HID), NPBF16)
            xg[:L] = xb[n0:n1]
            # node-major: [p, c, hid], node = c*128 + p
            x_nm[g] = xg.reshape(C, 128, HID).transpose(1, 0, 2)
            xg8 = np.zeros((gmax, HID), NPE4)
            xg8[:L] = x8[n0:n1]
            xT = np.ascontiguousarray(xg8[perm].T)  # [HID, gmax]
            x_fm[g] = xT.reshape(4, 128, F, 512).transpose(1, 2, 0, 3)
            oh = np.zeros((gmax, spg), np.float32)
            oh[np.arange(L), (batch[n0:n1] - s0).astype(np.int64)] = 1.0
            st_cs[g] = oh.reshape(C, 128, spg).transpose(1, 0, 2)
            segl = bounds[s0 + 1 : s0 + spg + 1] - bounds[s0:s0 + spg]
            lb2[:, g] = segl.astype(np.float32) * b2val - LN1E8
        st_sc = np.ascontiguousarray(st_cs.transpose(0, 1, 3, 2))
        in_maps.append(
            {
                "x_nm": x_nm,
                "x_fm": x_fm,
                "st_cs": np.ascontiguousarray(st_cs),
                "st_sc": st_sc,
                "w1": w1b,
                "b1": b1f,
                "w2v": w2v,
                "lb2": np.ascontiguousarray(lb2),
                "ident": ident,
            }
        )
    return in_maps, gmax, spg, b2val


def _run(inputs, trace=False, **run_kwargs):
    in_maps, gmax, spg, b2val = _prepare(
        inputs["x"], inputs["batch"], inputs["W1"], inputs["b1"],
        inputs["W2"], inputs["b2"], G,
    )
    nc = _build_graph(G, gmax, spg, b2val)
    res = run_bass_kernel_spmd(
        nc, in_maps, core_ids=list(range(NCORES)), trace=trace, **run_kwargs
    )
    out = np.concatenate([r["out"] for r in res.results], axis=0)
    return out.astype(np.float32), res


def kernel(**inputs) -> np.ndarray:
    out, _ = _run(inputs, trace=False)
    return out
